# revision 7
# baseline (speedup 1.0000x reference)
"""CGAMixer Trainium2 Bass kernel.

Strategy (data-parallel over batch, one sequence per NeuronCore):
  - Pre-phase (batched PE matmuls): q/k projections + l2-norm in transposed
    layout QT/KT [64, T]; v projection in interleaved column layout
    VCT [128, T, 2] (dm split into two 128-halves).
  - Sequential scan over T steps. Tiny per-step state updates with
    register-indexed dynamic gathers/scatters (sel = argmax via DVE
    max/max_index). Softmax is computed WITHOUT max-subtraction (scores*scale
    is tiny); invalid slots are masked additively with -1e9 injected into
    PSUM via a K=1 matmul. The centroid l2-norm uses
    rsqrt(0.82+0.18*(c.k)) = exp(-0.5*ln(.)) (c, k are unit vectors), keeping
    every per-step transcendental in one ACT table set (exp/ln).
  - z_t = softmax @ V is NOT computed in the scan. Since V_0 = 0 and each
    step does a rank-1 update, z_t = sum_{j<t} u_t[w_j] * dv_j. The scan
    stores scores columns (ScoresT), written slots (Wvals) and value deltas
    (DVT); a post-phase reconstructs Z with a handful of big matmuls.
  - Output y^T = Wo^T @ (Z/usum)^T + bo computed on device; host reassembles.
"""
import numpy as np

D_MODEL = 256
D_STATE = 64
M = 64
NEG = -1e9
T_FULL = 1024
B_FULL = 4
N_CORES = 8


# ---------------------------------------------------------------------------
# device program
# ---------------------------------------------------------------------------

def build_program(nc, T, scale):
    from concourse import bass, mybir
    from concourse.tile import TileContext

    F32 = mybir.dt.float32
    U32 = mybir.dt.uint32
    AF = mybir.ActivationFunctionType
    OP = mybir.AluOpType
    ds = bass.ds

    TP = ((T + 127) // 128) * 128          # padded T for 128-blocked post
    NJT = TP // 128
    N512 = [(i, min(512, T - i)) for i in range(0, T, 512)]   # free-dim chunks

    # ---- DRAM I/O -----------------------------------------------------
    d = {}
    def din(name, shape):
        d[name] = nc.dram_tensor(name, shape, F32, kind="ExternalInput").ap()
    din("xT2", [128, 2, T])
    din("wq2", [128, 2, D_STATE])
    din("wk2", [128, 2, D_STATE])
    din("wv2", [128, 2, D_MODEL])
    din("wo2", [128, 2, D_MODEL])
    din("bqc", [D_STATE, 1])
    din("bkc", [D_STATE, 1])
    din("bvc", [128, 2])
    din("boc", [128, 2])
    din("iota_row", [1, M])
    din("ones_row", [1, 128])
    din("negones_row", [1, 128])
    din("ones_col", [128, 1])
    din("one11", [1, 1])
    din("ident", [128, 128])
    din("iota64b", [M, TP])
    din("ltbase", [128, 2 * T])
    d_yT = nc.dram_tensor("yT", [2, 128, T], F32, kind="ExternalOutput").ap()

    with TileContext(nc) as tc:
        with tc.tile_pool(name="const", bufs=1) as cpool, \
             tc.tile_pool(name="big", bufs=1) as bpool, \
             tc.tile_pool(name="state", bufs=1) as spool, \
             tc.tile_pool(name="scr", bufs=3) as scr, \
             tc.tile_pool(name="colscr", bufs=3) as colscr, \
             tc.tile_pool(name="ps_sm", bufs=4, space="PSUM") as ps_sm, \
             tc.tile_pool(name="ps_big", bufs=2, space="PSUM") as ps_big:

            # ---- load constants/weights ------------------------------
            t_xT = bpool.tile([128, 2, T], F32)
            t_wq = cpool.tile([128, 2, D_STATE], F32)
            t_wk = cpool.tile([128, 2, D_STATE], F32)
            t_wv = cpool.tile([128, 2, D_MODEL], F32)
            t_wo = cpool.tile([128, 2, D_MODEL], F32)
            t_bqc = cpool.tile([D_STATE, 1], F32)
            t_bkc = cpool.tile([D_STATE, 1], F32)
            t_bvc = cpool.tile([128, 2], F32)
            t_boc = cpool.tile([128, 2], F32)
            t_iota = cpool.tile([1, M], F32)
            t_ones = cpool.tile([1, 128], F32)
            t_negones = cpool.tile([1, 128], F32)
            t_onesc = cpool.tile([128, 1], F32)
            t_one11 = cpool.tile([1, 1], F32)
            t_ident = cpool.tile([128, 128], F32)
            t_iota64b = bpool.tile([M, TP], F32)
            t_ltbase = bpool.tile([128, 2 * T], F32)
            for nm, tl in [("xT2", t_xT), ("wq2", t_wq), ("wk2", t_wk),
                           ("wv2", t_wv), ("wo2", t_wo), ("bqc", t_bqc),
                           ("bkc", t_bkc), ("bvc", t_bvc), ("boc", t_boc),
                           ("iota_row", t_iota), ("ones_row", t_ones),
                           ("negones_row", t_negones), ("ones_col", t_onesc),
                           ("one11", t_one11), ("ident", t_ident),
                           ("iota64b", t_iota64b), ("ltbase", t_ltbase)]:
                nc.sync.dma_start(tl[:], d[nm])

            # ---- projections -----------------------------------------
            def proj_qk(w, bias_col, out_name):
                """out [64, T] = l2norm_cols(w^T @ xT + bias)."""
                p = ps_big.tile([D_STATE, T], F32, tag="big2")
                for (o, nn) in N512:
                    for c in range(2):
                        nc.tensor.matmul(p[:, o:o + nn], w[:, c, :], t_xT[:, c, o:o + nn],
                                         start=(c == 0), stop=(c == 1))
                qb = bpool.tile([D_STATE, T], F32, tag=out_name + "_qb")
                nc.vector.tensor_scalar(qb[:], p[:], bias_col, None, OP.add)
                sq = bpool.tile([D_STATE, T], F32, tag=out_name + "_sq")
                nc.scalar.activation(sq[:], qb[:], AF.Square)
                pn = ps_big.tile([1, T], F32, tag="big2")
                for (o, nn) in N512:
                    nc.tensor.matmul(pn[:, o:o + nn], t_onesc[0:D_STATE, :],
                                     sq[:, o:o + nn], start=True, stop=True)
                nqc = scr.tile([1, T], F32, tag="nqc")
                nc.vector.tensor_scalar(nqc[:], pn[:], 1e-24, None, OP.max)
                lnq = scr.tile([1, T], F32, tag="lnq")
                nc.scalar.activation(lnq[:], nqc[:], AF.Ln)
                rq = scr.tile([1, T], F32, tag="rq")
                nc.scalar.activation(rq[:], lnq[:], AF.Exp, scale=-0.5)
                pb = ps_big.tile([D_STATE, T], F32, tag="big2")
                for (o, nn) in N512:
                    nc.tensor.matmul(pb[:, o:o + nn], t_ones[0:1, 0:D_STATE],
                                     rq[:, o:o + nn], start=True, stop=True)
                out = bpool.tile([D_STATE, T], F32, tag=out_name)
                nc.vector.tensor_tensor(out[:], qb[:], pb[:], OP.mult)
                return out

            t_QT = proj_qk(t_wq, t_bqc[:, 0:1], "QT")
            t_KT = proj_qk(t_wk, t_bkc[:, 0:1], "KT")

            t_VCT = bpool.tile([128, T, 2], F32)
            for h in range(2):
                pv = ps_big.tile([128, T], F32, tag="big2")
                for (o, nn) in N512:
                    for c in range(2):
                        nc.tensor.matmul(pv[:, o:o + nn],
                                         t_wv[:, c, h * 128:(h + 1) * 128],
                                         t_xT[:, c, o:o + nn],
                                         start=(c == 0), stop=(c == 1))
                nc.vector.tensor_scalar(t_VCT[:, :, h], pv[:],
                                        t_bvc[:, h:h + 1], None, OP.add)

            # ---- state init ------------------------------------------
            t_CT = spool.tile([D_STATE, M], F32)
            t_VT = spool.tile([128, 2 * M], F32)
            t_counts = spool.tile([1, M], F32)
            t_n = spool.tile([1, 1], F32)
            t_ScoresT = spool.tile([D_STATE, T], F32)
            t_DVT = spool.tile([128, TP, 2], F32)
            t_Wvals = spool.tile([1, TP], F32)
            nc.vector.memset(t_CT[:], 0.0)
            nc.vector.memset(t_VT[:], 0.0)
            nc.vector.memset(t_counts[:], 0.0)
            nc.vector.memset(t_ScoresT[:], 0.0)
            nc.vector.memset(t_DVT[:], 0.0)
            nc.vector.memset(t_Wvals[:], -1.0)
            t_c082 = cpool.tile([1, 1], F32)
            nc.vector.memset(t_c082[:], 0.82)
            t_ones12 = cpool.tile([1, 2], F32)
            nc.vector.memset(t_ones12[:], 1.0)

            # t = 0 hardcoded step
            nc.vector.tensor_copy(t_CT[:, 0:1], t_KT[:, 0:1])
            nc.vector.tensor_copy(t_VT[:, 0:2], t_VCT[:, 0, :])
            nc.vector.memset(t_counts[0:1, 0:1], 1.0)
            nc.vector.memset(t_n[:], 1.0)
            nc.vector.memset(t_Wvals[0:1, 0:1], 0.0)
            nc.vector.tensor_scalar(t_DVT[:, 0, :], t_VCT[:, 0, :], -1.0,
                                    None, OP.mult)

            # ---- registers -------------------------------------------
            selreg = nc.alloc_register(mybir.EngineType.Activation, "selreg")
            selv = bass.make_scalar_value(selreg, min_val=0, max_val=M - 1)
            wreg = nc.alloc_register(mybir.EngineType.DVE, "wreg")
            wv = bass.make_scalar_value(wreg, min_val=0, max_val=M - 1)

            # ---- the scan --------------------------------------------
            for t in range(1, T):
                p_sc = ps_sm.tile([1, M], F32, tag="sm")
                p_scol = ps_sm.tile([D_STATE, 2], F32, tag="sm")
                p_dck = ps_sm.tile([1, 1], F32, tag="sm")
                p_b = ps_sm.tile([128, 7], F32, tag="sm")
                p_ms = ps_sm.tile([1, 1], F32, tag="sm")

                mask = scr.tile([1, M], F32, tag="mask")
                scrw = scr.tile([1, M], F32, tag="scrw")
                max8 = scr.tile([1, 8], F32, tag="max8")
                sel8 = scr.tile([1, 8], U32, tag="sel8")
                selfl = scr.tile([1, 1], F32, tag="selfl")
                nMv = scr.tile([1, 1], F32, tag="nMv")
                r2 = scr.tile([1, 1], F32, tag="r2")
                rA = scr.tile([1, 1], F32, tag="rA")
                cnt1 = scr.tile([1, 1], F32, tag="cnt1")
                bnv = scr.tile([1, 1], F32, tag="bnv")
                asv = scr.tile([1, 1], F32, tag="asv")
                tk1 = scr.tile([1, 1], F32, tag="tk1")
                w1 = scr.tile([1, 1], F32, tag="w1")
                wf = scr.tile([1, 1], F32, tag="wf")
                wu = scr.tile([1, 1], U32, tag="wu")
                scal = scr.tile([1, 7], F32, tag="scal")
                csel = colscr.tile([D_STATE, 1], F32, tag="csel")
                t1c = colscr.tile([D_STATE, 1], F32, tag="t1c")
                cwr = colscr.tile([D_STATE, 1], F32, tag="cwr")
                vsel = colscr.tile([128, 2], F32, tag="vsel")
                diff = colscr.tile([128, 2], F32, tag="diff")
                d2 = colscr.tile([128, 2], F32, tag="d2")
                ssqp = colscr.tile([128, 1], F32, tag="ssqp")
                t2v = colscr.tile([128, 2], F32, tag="t2v")
                vwr = colscr.tile([128, 2], F32, tag="vwr")
                cntsel = scr.tile([1, 1], F32, tag="cntsel")
                lnv = scr.tile([1, 1], F32, tag="lnv")
                rs = scr.tile([1, 1], F32, tag="rs")

                SU, SV, SC, SK, CW, DU, DA = 0, 1, 2, 3, 4, 5, 6

                # scores = -1e9*(iota>=n) + q_t^T @ CT   (PSUM inject+accum)
                nc.vector.tensor_scalar(mask[:], t_iota[:], t_n[0:1, 0:1],
                                        NEG, OP.is_ge, OP.mult)
                nc.tensor.matmul(p_sc[:], t_one11[:], mask[:],
                                 start=True, stop=False)
                nc.tensor.matmul(p_sc[:], t_QT[:, t:t + 1], t_CT[:],
                                 start=False, stop=True)
                nc.vector.tensor_copy(scrw[:], p_sc[:])

                # argmax
                nc.vector.max(max8[:], scrw[:])
                nc.vector.max_index(sel8[:], max8[:], scrw[:])
                nc.vector.tensor_copy(selfl[:], sel8[0:1, 0:1])

                # store scores column (off critical path)
                nc.tensor.matmul(p_scol[:], scrw[:], t_ones12[:],
                                 start=True, stop=True)
                nc.vector.tensor_copy(t_ScoresT[:, t:t + 1], p_scol[:, 0:1])

                # gathers (ACT, via sel register)
                nc.scalar.reg_load(selreg, sel8[0:1, 0:1])
                nc.scalar.copy(csel[:], t_CT[:, ds(selv, 1)])
                nc.scalar.copy(vsel[:], t_VT[:, ds(selv * 2, 2)])
                nc.scalar.copy(cntsel[:], t_counts[0:1, ds(selv, 1)])

                # rs = rsqrt(0.82 + 0.18 * csel.k_t)  via exp(-0.5 ln())
                nc.tensor.matmul(p_dck[:], csel[:], t_KT[:, t:t + 1],
                                 start=True, stop=True)
                nc.scalar.activation(lnv[:], p_dck[:], AF.Ln,
                                     bias=t_c082[0:1, 0:1], scale=0.18)
                nc.scalar.activation(rs[:], lnv[:], AF.Exp, scale=-0.5)

                # residual ssq and flags
                nc.vector.tensor_tensor(diff[:], vsel[:], t_VCT[:, t, :],
                                        OP.subtract)
                nc.vector.tensor_tensor(d2[:], diff[:], diff[:], OP.mult)
                nc.vector.tensor_reduce(ssqp[:], d2[:], mybir.AxisListType.X,
                                        OP.add)
                nc.tensor.matmul(p_ms[:], ssqp[:], t_onesc[:],
                                 start=True, stop=True)
                nc.vector.tensor_scalar(nMv[:], t_n[:], float(M), None,
                                        OP.is_lt)
                nc.vector.tensor_scalar(r2[:], p_ms[:], float(D_MODEL), None,
                                        OP.is_gt)
                nc.vector.scalar_tensor_tensor(rA[:], max8[0:1, 0:1], 0.75,
                                               r2[:], OP.is_lt, OP.max)
                nc.vector.tensor_scalar(scal[0:1, DA:DA + 1], rA[:],
                                        nMv[0:1, 0:1], None, OP.mult)
                nc.vector.tensor_scalar(scal[0:1, DU:DU + 1],
                                        scal[0:1, DA:DA + 1], -1.0, 1.0,
                                        OP.mult, OP.add)
                du_ap = scal[0:1, DU:DU + 1]
                da_ap = scal[0:1, DA:DA + 1]

                # scalar soup
                nc.vector.tensor_scalar(cnt1[:], cntsel[:], 1.0, None, OP.add)
                nc.vector.reciprocal(bnv[:], cnt1[:])
                nc.vector.tensor_scalar(asv[:], cntsel[:], bnv[0:1, 0:1],
                                        None, OP.mult)
                nc.vector.tensor_scalar(scal[0:1, SU:SU + 1], asv[:], du_ap,
                                        None, OP.mult)
                nc.vector.scalar_tensor_tensor(scal[0:1, SV:SV + 1], bnv[:],
                                               du_ap, da_ap, OP.mult, OP.add)
                nc.vector.tensor_scalar(scal[0:1, SC:SC + 1], rs[:], du_ap,
                                        0.9, OP.mult, OP.mult)
                nc.vector.tensor_scalar(tk1[:], rs[:], du_ap, 0.1,
                                        OP.mult, OP.mult)
                nc.vector.tensor_tensor(scal[0:1, SK:SK + 1], tk1[:], da_ap,
                                        OP.add)
                nc.vector.scalar_tensor_tensor(scal[0:1, CW:CW + 1], cnt1[:],
                                               du_ap, da_ap, OP.mult, OP.add)

                # w = da*n + du*sel ; n += da
                nc.vector.tensor_scalar(w1[:], selfl[:], du_ap, None, OP.mult)
                nc.vector.scalar_tensor_tensor(wf[:], t_n[:], da_ap, w1[:],
                                               OP.mult, OP.add)
                nc.vector.tensor_copy(wu[:], wf[:])
                nc.vector.tensor_copy(t_Wvals[0:1, t:t + 1], wf[:])
                nc.vector.tensor_tensor(t_n[:], t_n[:], da_ap, OP.add)

                # broadcast the scalar bundle across 128 partitions
                nc.tensor.matmul(p_b[:], t_ones[0:1, :], scal[:],
                                 start=True, stop=True)

                # centroid write
                nc.vector.tensor_scalar(t1c[:], csel[:], p_b[0:D_STATE, SC:SC + 1],
                                        None, OP.mult)
                nc.vector.scalar_tensor_tensor(cwr[:], t_KT[:, t:t + 1],
                                               p_b[0:D_STATE, SK:SK + 1],
                                               t1c[:], OP.mult, OP.add)
                nc.vector.reg_load(wreg, wu[0:1, 0:1])
                nc.vector.tensor_copy(t_CT[:, ds(wv, 1)], cwr[:])

                # value write + delta
                nc.vector.tensor_scalar(t2v[:], t_VCT[:, t, :],
                                        p_b[:, SV:SV + 1], None, OP.mult)
                nc.vector.scalar_tensor_tensor(vwr[:], vsel[:],
                                               p_b[:, SU:SU + 1], t2v[:],
                                               OP.mult, OP.add)
                nc.vector.tensor_copy(t_VT[:, ds(wv * 2, 2)], vwr[:])
                nc.vector.scalar_tensor_tensor(t_DVT[:, t, :], vsel[:],
                                               p_b[:, DU:DU + 1], vwr[:],
                                               OP.mult, OP.subtract)

                # counts write
                nc.vector.tensor_copy(t_counts[0:1, ds(wv, 1)],
                                      scal[0:1, CW:CW + 1])

            # ---- post phase: Z reconstruction ------------------------
            t_UT = bpool.tile([D_STATE, T], F32)
            nc.scalar.activation(t_UT[:], t_ScoresT[:], AF.Exp,
                                 scale=float(scale))
            p_us = ps_big.tile([1, T], F32, tag="big2")
            for (o, nn) in N512:
                nc.tensor.matmul(p_us[:, o:o + nn], t_onesc[0:D_STATE, :],
                                 t_UT[:, o:o + nn], start=True, stop=True)
            lnu = scr.tile([1, T], F32, tag="lnu")
            nc.scalar.activation(lnu[:], p_us[:], AF.Ln)
            ru = scr.tile([1, T], F32, tag="ru")
            nc.scalar.activation(ru[:], lnu[:], AF.Exp, scale=-1.0)
            p_rub = ps_big.tile([128, T], F32, tag="big2")
            for (o, nn) in N512:
                nc.tensor.matmul(p_rub[:, o:o + nn], t_negones[0:1, :],
                                 ru[:, o:o + nn], start=True, stop=True)
            t_nrub = bpool.tile([128, T], F32)
            nc.vector.tensor_copy(t_nrub[:], p_rub[:])

            # OHT[m, j] = (Wvals[j] == m)
            p_wb = ps_big.tile([D_STATE, TP], F32, tag="big2")
            for o in range(0, TP, 512):
                nn = min(512, TP - o)
                nc.tensor.matmul(p_wb[:, o:o + nn], t_ones[0:1, 0:D_STATE],
                                 t_Wvals[0:1, o:o + nn], start=True, stop=True)
            t_OHT = bpool.tile([D_STATE, TP], F32)
            nc.vector.tensor_tensor(t_OHT[:], p_wb[:], t_iota64b[:],
                                    OP.is_equal)

            # DV row-major tiles from DVT columns
            t_DVrm = bpool.tile([128, NJT, D_MODEL], F32)
            for jt in range(NJT):
                for h in range(2):
                    p_tr = ps_big.tile([128, 128], F32, tag="big2")
                    nc.tensor.transpose(
                        p_tr[:], t_DVT[:, jt * 128:(jt + 1) * 128, h],
                        t_ident[:])
                    nc.vector.tensor_copy(
                        t_DVrm[:, jt, h * 128:(h + 1) * 128], p_tr[:])

            # ZT accumulation over j-tiles (accumulate in SBUF)
            t_ZT = bpool.tile([128, 2, T], F32)
            t_uwm = bpool.tile([128, T], F32)
            for jt in range(NJT):
                p_uw = ps_big.tile([128, T], F32, tag="big2")
                for (o, nn) in N512:
                    nc.tensor.matmul(p_uw[:, o:o + nn],
                                     t_OHT[:, jt * 128:(jt + 1) * 128],
                                     t_UT[:, o:o + nn], start=True, stop=True)
                nc.vector.tensor_tensor(
                    t_uwm[:], p_uw[:],
                    t_ltbase[:, T - 128 * jt:2 * T - 128 * jt], OP.mult)
                for h in range(2):
                    p_z = ps_big.tile([128, T], F32, tag="big2")
                    for (o, nn) in N512:
                        nc.tensor.matmul(p_z[:, o:o + nn],
                                         t_DVrm[:, jt, h * 128:(h + 1) * 128],
                                         t_uwm[:, o:o + nn],
                                         start=True, stop=True)
                    if jt == 0:
                        nc.vector.tensor_copy(t_ZT[:, h, :], p_z[:])
                    else:
                        nc.vector.tensor_tensor(t_ZT[:, h, :], t_ZT[:, h, :],
                                                p_z[:], OP.add)

            # normalize (with sign fold) and output projection
            t_ZnT = bpool.tile([128, 2, T], F32)
            nc.vector.tensor_tensor(t_ZnT[:, 0, :], t_ZT[:, 0, :], t_nrub[:], OP.mult)
            nc.vector.tensor_tensor(t_ZnT[:, 1, :], t_ZT[:, 1, :], t_nrub[:], OP.mult)
            for oh in range(2):
                p_y = ps_big.tile([128, T], F32, tag="big2")
                for (o, nn) in N512:
                    for h in range(2):
                        nc.tensor.matmul(p_y[:, o:o + nn],
                                         t_wo[:, h, oh * 128:(oh + 1) * 128],
                                         t_ZnT[:, h, o:o + nn],
                                         start=(h == 0), stop=(h == 1))
                t_y = bpool.tile([128, T], F32, tag="t_y")
                nc.vector.tensor_scalar(t_y[:], p_y[:], t_boc[:, oh:oh + 1], None,
                                        OP.add)
                nc.sync.dma_start(d_yT[oh], t_y[:])
    return nc


# ---------------------------------------------------------------------------
# host-side wrappers
# ---------------------------------------------------------------------------

def make_core_inputs(x_seq, Wq, bq, Wk, bk, Wv, bv, Wo, bo, T):
    """x_seq: [T, 256] one sequence -> in_map dict for one core."""
    TP = ((T + 127) // 128) * 128
    f = np.float32
    xT = np.ascontiguousarray(x_seq.T).astype(f)          # [256, T]
    ins = {
        "xT2": np.ascontiguousarray(xT.reshape(2, 128, T).transpose(1, 0, 2)),
        "wq2": np.ascontiguousarray(Wq.astype(f).reshape(2, 128, D_STATE).transpose(1, 0, 2)),
        "wk2": np.ascontiguousarray(Wk.astype(f).reshape(2, 128, D_STATE).transpose(1, 0, 2)),
        "wv2": np.ascontiguousarray(Wv.astype(f).reshape(2, 128, D_MODEL).transpose(1, 0, 2)),
        "wo2": np.ascontiguousarray(Wo.astype(f).reshape(2, 128, D_MODEL).transpose(1, 0, 2)),
        "bqc": bq.astype(f).reshape(D_STATE, 1),
        "bkc": bk.astype(f).reshape(D_STATE, 1),
        "bvc": np.ascontiguousarray(bv.astype(f).reshape(2, 128).T),
        "boc": np.ascontiguousarray(bo.astype(f).reshape(2, 128).T),
        "iota_row": np.arange(M, dtype=f).reshape(1, M),
        "ones_row": np.ones((1, 128), f),
        "negones_row": np.full((1, 128), -1.0, f),
        "ones_col": np.ones((128, 1), f),
        "one11": np.ones((1, 1), f),
        "ident": np.eye(128, dtype=f),
        "iota64b": np.broadcast_to(
            np.arange(M, dtype=f)[:, None], (M, TP)).copy(),
        "ltbase": (np.arange(2 * T, dtype=np.int64)[None, :] - T
                   > np.arange(128, dtype=np.int64)[:, None]).astype(f),
    }
    return ins


def host_post(yT):
    """yT [2, 128, T] -> y [T, 256]"""
    return np.concatenate([yT[0], yT[1]], axis=0).T


_COMPILED = {}


def _get_program(T, scale):
    key = (T, float(scale))
    if key not in _COMPILED:
        from concourse import bacc
        nc = bacc.Bacc("TRN2", target_bir_lowering=False, debug=False,
                       num_devices=N_CORES)
        build_program(nc, T, scale)
        nc.compile()
        from concourse.bass_interp import get_hw_module
        nc.m = get_hw_module(nc.m)
        _COMPILED[key] = nc
    return _COMPILED[key]


def kernel(x, Wq, bq, Wk, bk, Wv, bv, Wo, bo, logit_scale):
    from concourse import bass_utils
    x = np.asarray(x, dtype=np.float32)
    B, T, _ = x.shape
    scale = float(min(np.exp(np.float32(logit_scale)), np.float32(100.0)))
    nc = _get_program(T, scale)
    args = [np.asarray(a, np.float32) for a in
            (Wq, bq, Wk, bk, Wv, bv, Wo, bo)]
    in_maps = [make_core_inputs(x[b % B], *args, T) for b in range(N_CORES)]
    res = bass_utils.run_bass_kernel_spmd(
        nc, in_maps, core_ids=list(range(N_CORES)))
    ys = [host_post(res.results[b]["yT"]) for b in range(B)]
    return np.stack(ys).astype(np.float32)


if __name__ == "__main__":
    import jax
    jax.config.update("jax_platforms", "cpu")
    import reference
    inputs = {k: np.asarray(v) for k, v in reference.setup_inputs().items()}
    out = kernel(**inputs)
    print("kernel output", out.shape, out.dtype)
